# revision 1
# baseline (speedup 1.0000x reference)
"""Graph-transformer attention block on 8 Trainium2 NeuronCores.

Reference math (N=8192, D=256):
    Q = h @ Wq.T; K = h @ Wk.T; V = h @ Wv.T
    S = (1/16) * (Q @ K.T) * adj          # multiplicative 0/1 mask
    A = softmax(S, axis=1)                # exp(0)=1 for non-edges!
    X = A @ V

Sharding: row-shard queries across 8 cores (1024 q-rows each); K/V are
computed (replicated) on every core from the full h.

Per-core device algorithm. All score tiles live in TRANSPOSED layout
S_T[k, q] so the post-softmax tile is directly the lhsT of the A@V
matmul -- no on-device transposes anywhere. Using the exact identity
(adj is 0/1):
    P[k,q]    = exp(S*adj) = 1 + adj*(exp(S)-1)
    tmp[k,q]  = (exp(S)-1)*adj                     (one fused DVE op)
    U_T[e,q]  = sum_k tmp*V[k,e] + colsum_V[e]     (PE, PSUM-accumulated)
    rowsum[q] = N + sum_k tmp[k,q]                 (Pool acc + PE ones-reduce)
    X_T[e,q]  = (U_T + colsum_V)/rowsum
fp16 for matmul inputs (1 cyc/row on PE vs 4 for fp32; 3 more mantissa
bits than bf16), fp32 for PSUM/exp/divide.
"""

import os
import sys

import numpy as np

for _p in ("/opt/trn_rl_repo", "/root/.axon_site/_ro/trn_rl_repo"):
    if os.path.isdir(_p) and _p not in sys.path:
        sys.path.insert(0, _p)

N = 8192
D = 256
NCORES = 8
QPC = N // NCORES  # 1024 query rows per core
P = 128
SCALE = 1.0 / 16.0

_CACHE = {}


def build_program(n_k=N, n_q=QPC, SBUFS=3, EBUFS=3):
    """Build the SPMD per-core Bass program. n_k/n_q shrinkable for sim."""
    import concourse.bass as bass  # noqa: F401
    import concourse.tile as tile
    from concourse import bacc
    from concourse import mybir

    fp16 = mybir.dt.float16
    fp32 = mybir.dt.float32
    Alu = mybir.AluOpType
    Act = mybir.ActivationFunctionType

    n_kt = n_k // P                     # 128-row k tiles
    qw = min(n_q, 512)                  # q chunk width (PSUM bank limit)
    n_qc = n_q // qw
    kw = min(n_k, 512)
    n_kc = n_k // kw

    nc = bacc.Bacc(None)

    hT = nc.dram_tensor("hT", [D, n_k], fp16, kind="ExternalInput")
    hqT = nc.dram_tensor("hqT", [D, n_q], fp16, kind="ExternalInput")
    adjT = nc.dram_tensor("adjT", [n_k, n_q], fp16, kind="ExternalInput")
    wqT = nc.dram_tensor("wqT", [D, D], fp16, kind="ExternalInput")
    wkT = nc.dram_tensor("wkT", [D, D], fp16, kind="ExternalInput")
    wvT = nc.dram_tensor("wvT", [D, D], fp16, kind="ExternalInput")
    xT = nc.dram_tensor("xT", [D, n_q], fp32, kind="ExternalOutput")

    with tile.TileContext(nc) as tc:
        with (
            tc.tile_pool(name="const", bufs=1) as cpool,
            tc.tile_pool(name="stream", bufs=SBUFS) as spool,
            tc.tile_pool(name="epool", bufs=EBUFS) as epool,
            tc.tile_pool(name="upsum", bufs=1, space="PSUM") as upsum,
        ):
            # ---- constants ----
            w_sb = {}
            for name, dram in (("q", wqT), ("k", wkT), ("v", wvT)):
                for ch in range(2):
                    t = cpool.tile([P, D], fp16, tag=f"w{name}{ch}", name=f"w{name}{ch}")
                    nc.sync.dma_start(out=t[:], in_=dram[ch * P:(ch + 1) * P, :])
                    w_sb[name, ch] = t
            hqT_sb = []
            for ch in range(2):
                t = cpool.tile([P, n_q], fp16, tag=f"hqT{ch}", name=f"hqT{ch}")
                nc.sync.dma_start(out=t[:], in_=hqT[ch * P:(ch + 1) * P, :])
                hqT_sb.append(t)
            NHC = 4 if n_k % (4 * 512) == 0 else 1
            hcw = n_k // NHC                      # hT chunk width
            hT_sb = []                            # hT_sb[ch][cc] -> [P, hcw]
            for ch in range(2):
                chunks = []
                for cc in range(NHC):
                    t = cpool.tile([P, hcw], fp16, tag=f"hT{ch}_{cc}",
                                   name=f"hT{ch}_{cc}")
                    nc.sync.dma_start(
                        out=t[:],
                        in_=hT[ch * P:(ch + 1) * P, cc * hcw:(cc + 1) * hcw],
                    )
                    chunks.append(t)
                hT_sb.append(chunks)
            ones16 = cpool.tile([P, 1], fp16, tag="ones16")
            nc.gpsimd.memset(ones16[:], 1.0)
            onesc = cpool.tile([P, 1], fp32, tag="onesc")
            nc.gpsimd.memset(onesc[:], 1.0)
            ones_row = cpool.tile([1, P], fp32, tag="ones_row")
            nc.gpsimd.memset(ones_row[:], 1.0)
            acc_d = cpool.tile([P, n_q], fp16, tag="acc_d")
            nc.gpsimd.memset(acc_d[:], 0.0)
            acc_p = cpool.tile([P, n_q], fp16, tag="acc_p")
            nc.gpsimd.memset(acc_p[:], 0.0)

            kT_sb = [cpool.tile([P, n_k], fp16, tag=f"kT{dh}", name=f"kT{dh}") for dh in range(2)]
            qT_sb = [cpool.tile([P, n_q], fp16, tag=f"qT{dh}", name=f"qT{dh}") for dh in range(2)]
            v_sb = cpool.tile([P, n_kt * D], fp16, tag="v_sb")
            cs_sb = [cpool.tile([P, 1], fp32, tag=f"cs{eh}", name=f"cs{eh}") for eh in range(2)]

            # ---- prologue: projections ----
            with tc.tile_pool(name="ppsum", bufs=2, space="PSUM") as ppsum:
                # Q_T[dh][128, n_q]
                for dh in range(2):
                    for qc in range(n_qc):
                        pq = ppsum.tile([P, qw], fp32, tag="pp", name="pq")
                        for ch in range(2):
                            nc.tensor.matmul(
                                pq[:],
                                w_sb["q", ch][:, dh * P:(dh + 1) * P],
                                hqT_sb[ch][:, qc * qw:(qc + 1) * qw],
                                start=(ch == 0),
                                stop=(ch == 1),
                            )
                        nc.scalar.activation(
                            qT_sb[dh][:, qc * qw:(qc + 1) * qw], pq[:], Act.Copy
                        )
                # K_hT[dh][128, n_k]
                for dh in range(2):
                    for kc in range(n_kc):
                        pk = ppsum.tile([P, kw], fp32, tag="pp", name="pk")
                        for ch in range(2):
                            nc.tensor.matmul(
                                pk[:],
                                w_sb["k", ch][:, dh * P:(dh + 1) * P],
                                hT_sb[ch][(kc * kw) // hcw][
                                    :, (kc * kw) % hcw:(kc * kw) % hcw + kw],
                                start=(ch == 0),
                                stop=(ch == 1),
                            )
                        nc.vector.tensor_copy(
                            kT_sb[dh][:, kc * kw:(kc + 1) * kw], pk[:]
                        )
                # V[k, e] tiles
                for kt in range(n_kt):
                    pv = ppsum.tile([P, D], fp32, tag="pp", name="pv")
                    for ch in range(2):
                        nc.tensor.matmul(
                            pv[:],
                            hT_sb[ch][(kt * P) // hcw][
                                :, (kt * P) % hcw:(kt * P) % hcw + P],
                            w_sb["v", ch][:],
                            start=(ch == 0),
                            stop=(ch == 1),
                        )
                    nc.scalar.activation(v_sb[:, kt * D:(kt + 1) * D], pv[:], Act.Copy)
                # colsum_V in per-partition column form [e-half, 1]
                for eh in range(2):
                    pcs = ppsum.tile([P, 1], fp32, tag="pcs", name=f"pcs{eh}")
                    for kt in range(n_kt):
                        nc.tensor.matmul(
                            pcs[:],
                            v_sb[:, kt * D + eh * P:kt * D + (eh + 1) * P],
                            ones16[:],
                            start=(kt == 0),
                            stop=(kt == n_kt - 1),
                        )
                    nc.scalar.activation(cs_sb[eh][:], pcs[:], Act.Copy)

            # ---- main loop over k tiles ----
            pu = [upsum.tile([P, n_q], fp32, tag=f"pu{eh}", name=f"pu{eh}") for eh in range(2)]
            with tc.tile_pool(name="spsum", bufs=2, space="PSUM") as spsum:
                for t in range(n_kt):
                    adj_t = spool.tile([P, n_q], fp16, tag="adj")
                    nc.sync.dma_start(out=adj_t[:], in_=adjT[t * P:(t + 1) * P, :])
                    ps = spsum.tile([P, n_q], fp32, tag="ps")
                    for dh in range(2):
                        for qc in range(n_qc):
                            nc.tensor.matmul(
                                ps[:, qc * qw:(qc + 1) * qw],
                                kT_sb[dh][:, t * P:(t + 1) * P],
                                qT_sb[dh][:, qc * qw:(qc + 1) * qw],
                                start=(dh == 0),
                                stop=(dh == 1),
                            )
                    e_t = epool.tile([P, n_q], fp16, tag="e")
                    nc.scalar.activation(e_t[:], ps[:], Act.Exp, scale=SCALE)
                    em1 = epool.tile([P, n_q], fp16, tag="em1")
                    nc.vector.tensor_scalar_sub(em1[:], e_t[:], onesc[:, 0:1])
                    tmp = spool.tile([P, n_q], fp16, tag="tmp")
                    nc.vector.tensor_mul(tmp[:], em1[:], adj_t[:])
                    if t % 2 == 0:
                        nc.vector.tensor_add(acc_d[:], acc_d[:], tmp[:])
                    else:
                        nc.vector.tensor_add(acc_p[:], acc_p[:], tmp[:])
                    for eh in range(2):
                        for qc in range(n_qc):
                            nc.tensor.matmul(
                                pu[eh][:, qc * qw:(qc + 1) * qw],
                                v_sb[:, t * D + eh * P:t * D + (eh + 1) * P],
                                tmp[:, qc * qw:(qc + 1) * qw],
                                start=(t == 0),
                                stop=(t == n_kt - 1),
                            )

            # ---- tail: denominator + divide ----
            with tc.tile_pool(name="tpsum", bufs=1, space="PSUM") as tpsum:
                pr = tpsum.tile([1, n_q], fp32, tag="pr")
                for qc in range(n_qc):
                    nc.tensor.matmul(
                        pr[:, qc * qw:(qc + 1) * qw],
                        ones16[:],
                        acc_d[:, qc * qw:(qc + 1) * qw],
                        start=True,
                        stop=False,
                    )
                    nc.tensor.matmul(
                        pr[:, qc * qw:(qc + 1) * qw],
                        ones16[:],
                        acc_p[:, qc * qw:(qc + 1) * qw],
                        start=False,
                        stop=True,
                    )
                rs = cpool.tile([1, n_q], fp32, tag="rs")
                nc.vector.tensor_scalar_add(rs[:], pr[:], float(n_k))
                rc = cpool.tile([1, n_q], fp32, tag="rc")
                nc.vector.reciprocal(rc[:], rs[:])
                # broadcast 1/rowsum to all 128 partitions via K=1 matmul
                pb = tpsum.tile([P, n_q], fp32, tag="pb")
                for qc in range(n_qc):
                    nc.tensor.matmul(
                        pb[:, qc * qw:(qc + 1) * qw],
                        ones_row[:],
                        rc[:, qc * qw:(qc + 1) * qw],
                        start=True,
                        stop=True,
                    )
                pb_sb = cpool.tile([P, n_q], fp32, tag="pb_sb")
                nc.scalar.activation(pb_sb[:], pb[:], Act.Copy)
                for eh in range(2):
                    x_sb = cpool.tile([P, n_q], fp32, tag=f"x{eh}", name=f"x{eh}")
                    nc.vector.scalar_tensor_tensor(
                        x_sb[:], pu[eh][:], cs_sb[eh][:, 0:1], pb_sb[:],
                        op0=Alu.add, op1=Alu.mult,
                    )
                    nc.sync.dma_start(out=xT[eh * P:(eh + 1) * P, :], in_=x_sb[:])

    nc.finalize()
    return nc


def _host_prep(adj, h, Wq, Wk, Wv):
    hT16 = np.ascontiguousarray(h.T.astype(np.float16))
    wq16 = np.ascontiguousarray(Wq.T.astype(np.float16))
    wk16 = np.ascontiguousarray(Wk.T.astype(np.float16))
    wv16 = np.ascontiguousarray(Wv.T.astype(np.float16))
    adjT16 = np.ascontiguousarray(adj.T.astype(np.float16))
    in_maps = []
    for c in range(NCORES):
        in_maps.append({
            "hT": hT16,
            "hqT": np.ascontiguousarray(hT16[:, c * QPC:(c + 1) * QPC]),
            "adjT": np.ascontiguousarray(adjT16[:, c * QPC:(c + 1) * QPC]),
            "wqT": wq16,
            "wkT": wk16,
            "wvT": wv16,
        })
    return in_maps


def kernel(adj, h, Wq, Wk, Wv, _trace=False):
    from concourse.bass_utils import run_bass_kernel_spmd

    if "nc" not in _CACHE:
        _CACHE["nc"] = build_program()
    nc = _CACHE["nc"]
    in_maps = _host_prep(adj, h, Wq, Wk, Wv)
    res = run_bass_kernel_spmd(nc, in_maps, list(range(NCORES)), trace=_trace)
    out = np.empty([N, D], np.float32)
    for c in range(NCORES):
        out[c * QPC:(c + 1) * QPC, :] = np.asarray(
            res.results[c]["xT"], np.float32
        ).T
    if _trace:
        return out, res
    return out



# revision 19
# speedup vs baseline: 1.0931x; 1.0931x over previous
"""Graph-transformer attention block on 8 Trainium2 NeuronCores.

Reference math (N=8192, D=256):
    Q = h @ Wq.T; K = h @ Wk.T; V = h @ Wv.T
    S = (1/16) * (Q @ K.T) * adj          # multiplicative 0/1 mask
    A = softmax(S, axis=1)                # exp(0)=1 for non-edges!
    X = A @ V

Sharding: row-shard queries across 8 cores (1024 q-rows each); K/V math
is replicated per core.

Per-core algorithm (k-major transposed tiles [k_part, q_free]). Both
weight projections are folded OUT of the O(N^2) path:
    S^T = h ox Zq            Zq = Wk^T Wq hq^T  [256, n_q] tiny precompute
    e   = exp(S/16)          ACT, fp16
    tmp = (e - 1) * adj      ONE fused DVE op (affine_mul_reduce) -> fp16
    M   = h^T ox tmp         [256, n_q] PSUM-accumulated over k tiles (fp16)
    U^T = csV + Wv^T @ M     tiny tail matmul (Wv never touches O(N^2))
    rowsum = N + ones ox tmp8(e5m2 shadow)   fp8 DoubleRow, tail
    X^T = U^T / rowsum

Precision design (rel-err budget 2e-2, achieved ~5e-3): the score matmul
runs as THREE fp8e4 DoubleRow streams with error feedback
(h_hi Z_hi + h_lo Z_hi + h_hi Z_lo, where *_lo = fp8 of the fp8
quantization residual) -- fp8 speed at ~fp16 accuracy. exp/tmp/M stay
fp16: the exp'd scores are heavy-tailed (sigma(S)=1.7, values to e^9),
so fp8's ~4-7% relative noise on tmp or on M's h operand alone costs
2-4% output error (measured). The rowsum tolerates an e5m2 shadow copy
of tmp (0.5%). csV = (ones^T h) @ Wv^T is an exact fp16/fp32 side path
so quantization never touches the dense colsum term that dominates X.
"""

import os
import sys

import numpy as np

for _p in ("/opt/trn_rl_repo", "/root/.axon_site/_ro/trn_rl_repo"):
    if os.path.isdir(_p) and _p not in sys.path:
        sys.path.insert(0, _p)

N = 8192
D = 256
NCORES = 8
QPC = N // NCORES  # 1024 query rows per core
P = 128
SCALE = 1.0 / 16.0

_CACHE = {}


def build_program(n_k=N, n_q=QPC):
    """Build the SPMD per-core Bass program. n_k/n_q shrinkable for tests.

    Requires n_k % 256 == 0 and n_q in {256, 512, multiples of 512}.
    """
    import concourse.bass as bass  # noqa: F401
    import concourse.tile as tile
    from concourse import bacc
    from concourse import mybir

    fp16 = mybir.dt.float16
    fp32 = mybir.dt.float32
    fp8e4 = mybir.dt.float8e4
    fp8e5 = mybir.dt.float8e5
    Alu = mybir.AluOpType
    Act = mybir.ActivationFunctionType
    DR = mybir.MatmulPerfMode.DoubleRow

    n_kt = n_k // P                     # 128-row k tiles
    n_pair = n_kt // 2                  # DoubleRow processes k-tile pairs
    qw = min(n_q, 512)                  # q chunk width (PSUM bank limit)
    n_qc = n_q // qw

    nc = bacc.Bacc(None)

    # DRAM inputs (host pre-packed, see _host_prep)
    adjp8 = nc.dram_tensor("adjp8", [n_pair * P, 2 * n_q], fp8e4, kind="ExternalInput")
    hT8hid = nc.dram_tensor("hT8hid", [P, 2 * n_k], fp8e4, kind="ExternalInput")
    hT8lod = nc.dram_tensor("hT8lod", [P, 2 * n_k], fp8e4, kind="ExternalInput")
    h16od = nc.dram_tensor("h16od", [P, n_kt * D], fp16, kind="ExternalInput")
    hqT16d = nc.dram_tensor("hqT16d", [D, n_q], fp16, kind="ExternalInput")
    wq16d = nc.dram_tensor("wq16d", [D, D], fp16, kind="ExternalInput")   # Wq.T
    wk16d = nc.dram_tensor("wk16d", [D, D], fp16, kind="ExternalInput")   # Wk
    wvT16d = nc.dram_tensor("wvT16d", [D, D], fp16, kind="ExternalInput")  # Wv.T
    xT = nc.dram_tensor("xT", [D, n_q], fp32, kind="ExternalOutput")

    def r2(ap):
        # [128, (2*w)] -> [128, 2, w]
        return ap.rearrange("p (i w) -> p i w", i=2)

    with tile.TileContext(nc) as tc:
        with (
            tc.tile_pool(name="const", bufs=1) as cpool,
            tc.tile_pool(name="stream", bufs=3) as spool,
        ):
            # ---- resident SBUF tensors ----
            hT8hi = cpool.tile([P, 2 * n_k], fp8e4, tag="hT8hi")
            nc.sync.dma_start(out=hT8hi[:], in_=hT8hid[:, :])
            hT8lo = cpool.tile([P, 2 * n_k], fp8e4, tag="hT8lo")
            nc.sync.dma_start(out=hT8lo[:], in_=hT8lod[:, :])
            h16o = cpool.tile([P, n_kt * D], fp16, tag="h16o")
            nc.sync.dma_start(out=h16o[:], in_=h16od[:, :])
            hqT_sb = []
            for ch in range(2):
                t = cpool.tile([P, n_q], fp16, tag=f"hqT{ch}", name=f"hqT{ch}")
                nc.sync.dma_start(out=t[:], in_=hqT16d[ch * P:(ch + 1) * P, :])
                hqT_sb.append(t)
            wq_sb = []
            wk_sb = []
            wvT_sb = []
            for ch in range(2):
                t = cpool.tile([P, D], fp16, tag=f"wq{ch}", name=f"wq{ch}")
                nc.sync.dma_start(out=t[:], in_=wq16d[ch * P:(ch + 1) * P, :])
                wq_sb.append(t)
                t = cpool.tile([P, D], fp16, tag=f"wk{ch}", name=f"wk{ch}")
                nc.sync.dma_start(out=t[:], in_=wk16d[ch * P:(ch + 1) * P, :])
                wk_sb.append(t)
                t = cpool.tile([P, D], fp16, tag=f"wvT{ch}", name=f"wvT{ch}")
                nc.sync.dma_start(out=t[:], in_=wvT16d[ch * P:(ch + 1) * P, :])
                wvT_sb.append(t)

            Zq8hi = cpool.tile([P, 2 * n_q], fp8e4, tag="Zq8hi")
            Zq8lo = cpool.tile([P, 2 * n_q], fp8e4, tag="Zq8lo")
            tmp8 = cpool.tile([P, n_kt * n_q], fp8e5, tag="tmp8")
            QT16 = [cpool.tile([P, n_q], fp16, tag=f"QT{eh}", name=f"QT{eh}")
                    for eh in range(2)]
            M16 = [cpool.tile([P, n_q], fp16, tag=f"M16{dh}", name=f"M16{dh}")
                   for dh in range(2)]
            csh_sb = [cpool.tile([P, 1], fp16, tag=f"csh{dh}", name=f"csh{dh}")
                      for dh in range(2)]
            cv_row = cpool.tile([1, D], fp16, tag="cv_row")
            csV_sb = [cpool.tile([P, 1], fp32, tag=f"csV{eh}", name=f"csV{eh}")
                      for eh in range(2)]
            amr_scratch = cpool.tile([P, max(n_pair, 1)], fp32, tag="amr_scr")
            ones16 = cpool.tile([P, 1], fp16, tag="ones16")
            nc.gpsimd.memset(ones16[:], 1.0)
            # dual-fp8 ldweights wants the pair-dim stride 16B-aligned, so
            # lay the two ones-columns 16 bytes apart
            ones8 = cpool.tile([P, 32], fp8e5, tag="ones8")
            nc.gpsimd.memset(ones8[:], 1.0)
            ones16_row = cpool.tile([1, 1], fp16, tag="ones16r")
            nc.gpsimd.memset(ones16_row[:], 1.0)
            ones16_bc = cpool.tile([1, P], fp16, tag="ones16bc")
            nc.gpsimd.memset(ones16_bc[:], 1.0)

            # ---- prologue: QT, Zq (hi+lo fp8 split), csh, csV ----
            with tc.tile_pool(name="ppsum", bufs=2, space="PSUM") as ppsum:
                # QT[e',q] = sum_d Wq[e',d] hq^T[d,q]
                for eh in range(2):
                    pq = ppsum.tile([P, n_q], fp32, tag="pp", name=f"pq{eh}")
                    for qc in range(n_qc):
                        for ch in range(2):
                            nc.tensor.matmul(
                                pq[:, qc * qw:(qc + 1) * qw],
                                wq_sb[ch][:, eh * P:(eh + 1) * P],
                                hqT_sb[ch][:, qc * qw:(qc + 1) * qw],
                                start=(ch == 0), stop=(ch == 1),
                            )
                    nc.vector.tensor_copy(QT16[eh][:], pq[:])
                # Zq[d,q] = sum_e' Wk[e',d] QT[e',q] -> fp8 hi + fp8 residual
                for dh in range(2):
                    pz = ppsum.tile([P, n_q], fp32, tag="pp", name=f"pz{dh}")
                    for qc in range(n_qc):
                        for eh in range(2):
                            nc.tensor.matmul(
                                pz[:, qc * qw:(qc + 1) * qw],
                                wk_sb[eh][:, dh * P:(dh + 1) * P],
                                QT16[eh][:, qc * qw:(qc + 1) * qw],
                                start=(eh == 0), stop=(eh == 1),
                            )
                    zhi = Zq8hi[:, dh * n_q:(dh + 1) * n_q]
                    nc.vector.tensor_copy(zhi, pz[:])
                    nc.vector.tensor_tensor(
                        Zq8lo[:, dh * n_q:(dh + 1) * n_q], pz[:], zhi,
                        op=Alu.subtract,
                    )
                # csh[d] = sum_k h[k,d]  (exact fp16 inputs, fp32 PSUM)
                csh_ps = [ppsum.tile([P, 1], fp32, tag=f"pcs{dh}", bufs=1,
                                     name=f"pcs{dh}")
                          for dh in range(2)]
                for kt in range(n_kt):
                    for dh in range(2):
                        nc.tensor.matmul(
                            csh_ps[dh][:],
                            h16o[:, kt * D + dh * P: kt * D + dh * P + P],
                            ones16[:],
                            start=(kt == 0), stop=(kt == n_kt - 1),
                        )
                for dh in range(2):
                    nc.vector.tensor_copy(csh_sb[dh][:], csh_ps[dh][:])
                # csV[e] = sum_d csh[d] Wv[e,d] = csh @ Wv^T   [1, 256]
                pcv = ppsum.tile([1, D], fp32, tag="pcv", bufs=1)
                for dh in range(2):
                    nc.tensor.matmul(
                        pcv[:], csh_sb[dh][:], wvT_sb[dh][:],
                        start=(dh == 0), stop=(dh == 1),
                    )
                nc.vector.tensor_copy(cv_row[:], pcv[:])
                # transpose [1,256] -> two [128,1] columns
                for eh in range(2):
                    pct = ppsum.tile([P, 1], fp32, tag="pct", bufs=1,
                                     name=f"pct{eh}")
                    nc.tensor.matmul(
                        pct[:], cv_row[0:1, eh * P:(eh + 1) * P],
                        ones16_row[:], start=True, stop=True,
                    )
                    nc.vector.tensor_copy(csV_sb[eh][:], pct[:])

            # ---- main loop over k-tile pairs ----
            hHr = r2(hT8hi[:])                       # [128, 2, n_k]
            hLr = r2(hT8lo[:])
            zHr = r2(Zq8hi[:])                       # [128, 2, n_q]
            zLr = r2(Zq8lo[:])
            ones8r = ones8[:].rearrange(
                "p (i w) -> p i w", i=2)[:, :, 0:1]  # [128, 2, 1], step 16

            with tc.tile_pool(name="upsum", bufs=1, space="PSUM") as upsum:
                pm = [upsum.tile([P, n_q], fp32, tag=f"pm{dh}", name=f"pm{dh}")
                      for dh in range(2)]
                with tc.tile_pool(name="spsum", bufs=2, space="PSUM") as spsum:
                    for t in range(n_pair):
                        adj_t = spool.tile([P, 2 * n_q], fp8e4, tag="adj")
                        nc.sync.dma_start(
                            out=adj_t[:], in_=adjp8[t * P:(t + 1) * P, :])
                        e_pair = spool.tile([P, 2 * n_q], fp16, tag="e")
                        for i in range(2):
                            kt = 2 * t + i
                            ks = slice(kt * P, (kt + 1) * P)
                            ps = spsum.tile([P, n_q], fp32, tag="ps",
                                            name=f"ps{kt}")
                            for qc in range(n_qc):
                                qs = slice(qc * qw, (qc + 1) * qw)
                                nc.tensor.matmul(
                                    ps[:, qs], hHr[:, :, ks], zHr[:, :, qs],
                                    start=True, stop=False, perf_mode=DR,
                                )
                                nc.tensor.matmul(
                                    ps[:, qs], hLr[:, :, ks], zHr[:, :, qs],
                                    start=False, stop=False, perf_mode=DR,
                                )
                                nc.tensor.matmul(
                                    ps[:, qs], hHr[:, :, ks], zLr[:, :, qs],
                                    start=False, stop=True, perf_mode=DR,
                                )
                            nc.scalar.activation(
                                e_pair[:, i * n_q:(i + 1) * n_q], ps[:],
                                Act.Exp, scale=SCALE,
                            )
                        # tmp = (e - 1) * adj  -> fp16, one fused DVE op
                        tmp16 = spool.tile([P, 2 * n_q], fp16, tag="tmp")
                        nc.vector.affine_mul_reduce(
                            tmp16[:], amr_scratch[:, t:t + 1],
                            e_pair[:], adj_t[:], 1.0, -1.0,
                        )
                        # e5m2 shadow for the tail rowsum (DVE/Pool split)
                        shadow = tmp8[:, t * 2 * n_q:(t + 1) * 2 * n_q]
                        if t % 4 == 0:
                            nc.vector.tensor_copy(shadow, tmp16[:])
                        else:
                            nc.gpsimd.tensor_copy(shadow, tmp16[:])
                        # M[d,q] += sum_k h[k,d] tmp[k,q]   (fp16)
                        for i in range(2):
                            kt = 2 * t + i
                            for dh in range(2):
                                for qc in range(n_qc):
                                    nc.tensor.matmul(
                                        pm[dh][:, qc * qw:(qc + 1) * qw],
                                        h16o[:, kt * D + dh * P:
                                             kt * D + dh * P + P],
                                        tmp16[:, i * n_q + qc * qw:
                                              i * n_q + (qc + 1) * qw],
                                        start=(t == 0 and i == 0),
                                        stop=(t == n_pair - 1 and i == 1),
                                    )

                for dh in range(2):
                    nc.vector.tensor_copy(M16[dh][:], pm[dh][:])

            # ---- tail: rowsum, U^T = csV + Wv^T M, divide ----
            with tc.tile_pool(name="tpsum", bufs=1, space="PSUM") as tpsum:
                pr = tpsum.tile([1, n_q], fp32, tag="pr")
                for t in range(n_pair):
                    tmp8r = r2(tmp8[:, t * 2 * n_q:(t + 1) * 2 * n_q])
                    for qc in range(n_qc):
                        nc.tensor.matmul(
                            pr[0:1, qc * qw:(qc + 1) * qw],
                            ones8r[:],
                            tmp8r[:, :, qc * qw:(qc + 1) * qw],
                            start=(t == 0), stop=(t == n_pair - 1),
                            perf_mode=DR,
                        )
                rs = cpool.tile([1, n_q], fp32, tag="rs")
                nc.vector.tensor_scalar_add(rs[:], pr[:], float(n_k))
                rc = cpool.tile([1, n_q], fp32, tag="rc")
                nc.vector.reciprocal(rc[:], rs[:])
                rc16 = cpool.tile([1, n_q], fp16, tag="rc16")
                nc.vector.tensor_copy(rc16[:], rc[:])
                # broadcast 1/rowsum to 128 partitions via K=1 matmul
                pb = tpsum.tile([P, n_q], fp32, tag="pb")
                for qc in range(n_qc):
                    nc.tensor.matmul(
                        pb[:, qc * qw:(qc + 1) * qw],
                        ones16_bc[:],
                        rc16[0:1, qc * qw:(qc + 1) * qw],
                        start=True, stop=True,
                    )
                pb_sb = cpool.tile([P, n_q], fp32, tag="pb_sb")
                nc.scalar.activation(pb_sb[:], pb[:], Act.Copy)
                # U^T[e,q] = sum_d Wv^T[d,e] M[d,q]
                for eh in range(2):
                    pu = tpsum.tile([P, n_q], fp32, tag="pu", name=f"pu{eh}")
                    for qc in range(n_qc):
                        for dh in range(2):
                            nc.tensor.matmul(
                                pu[:, qc * qw:(qc + 1) * qw],
                                wvT_sb[dh][:, eh * P:(eh + 1) * P],
                                M16[dh][:, qc * qw:(qc + 1) * qw],
                                start=(dh == 0), stop=(dh == 1),
                            )
                    x_sb = cpool.tile([P, n_q], fp32, tag=f"x{eh}",
                                      name=f"x{eh}")
                    nc.vector.scalar_tensor_tensor(
                        x_sb[:], pu[:], csV_sb[eh][:, 0:1], pb_sb[:],
                        op0=Alu.add, op1=Alu.mult,
                    )
                    nc.sync.dma_start(out=xT[eh * P:(eh + 1) * P, :],
                                      in_=x_sb[:])

    nc.finalize()
    return nc


def _host_prep(adj, h, Wq, Wk, Wv, n_k=N, n_q=QPC, ncores=NCORES):
    import ml_dtypes

    e4 = ml_dtypes.float8_e4m3
    n_kt = n_k // P
    n_pair = n_kt // 2

    h32 = np.ascontiguousarray(h, dtype=np.float32)
    hT32 = np.ascontiguousarray(h32.T)                            # [D, n_k]
    # error-feedback fp8 split of h^T, packed dh-major [128, 2*n_k]
    hT8hi32 = hT32.astype(e4).astype(np.float32)
    hT8lo = (hT32 - hT8hi32).astype(e4)
    hT8hi = hT8hi32.astype(e4)

    def pack_dh(a):
        return np.ascontiguousarray(
            a.reshape(2, P, n_k).transpose(1, 0, 2).reshape(P, 2 * n_k))

    hT8hi = pack_dh(hT8hi)
    hT8lo = pack_dh(hT8lo)
    # h in k-tile-major fp16: h16o[p, kt*256 + d] = h[kt*128+p, d]
    h16o = np.ascontiguousarray(
        h32.astype(np.float16).reshape(n_kt, P, D).transpose(1, 0, 2)
        .reshape(P, n_kt * D))
    hT16 = np.ascontiguousarray(hT32.astype(np.float16))
    wq16 = np.ascontiguousarray(Wq.T.astype(np.float16))          # Wq.T
    wk16 = np.ascontiguousarray(Wk.astype(np.float16))            # Wk
    wvT16 = np.ascontiguousarray(Wv.T.astype(np.float16))         # Wv.T

    adjT = adj.T  # [k, q] orientation (full)
    in_maps = []
    for c in range(ncores):
        q0 = c * n_q
        adjTc = adjT[:n_k, q0:q0 + n_q]
        # [n_pair*128, 2*n_q]: row (t*128+p), col (i*n_q+q) = adjT[(2t+i)*128+p, q]
        adjp8 = np.ascontiguousarray(
            adjTc.reshape(n_pair, 2, P, n_q).transpose(0, 2, 1, 3)
            .reshape(n_pair * P, 2 * n_q).astype(e4))
        in_maps.append({
            "adjp8": adjp8,
            "hT8hid": hT8hi,
            "hT8lod": hT8lo,
            "h16od": h16o,
            "hqT16d": np.ascontiguousarray(hT16[:, q0:q0 + n_q]),
            "wq16d": wq16,
            "wk16d": wk16,
            "wvT16d": wvT16,
        })
    return in_maps


def kernel(adj, h, Wq, Wk, Wv, _trace=False):
    from concourse.bass_utils import run_bass_kernel_spmd

    if "nc" not in _CACHE:
        _CACHE["nc"] = build_program()
    nc = _CACHE["nc"]
    in_maps = _host_prep(adj, h, Wq, Wk, Wv)
    res = run_bass_kernel_spmd(nc, in_maps, list(range(NCORES)), trace=_trace)
    out = np.empty([N, D], np.float32)
    for c in range(NCORES):
        out[c * QPC:(c + 1) * QPC, :] = np.asarray(
            res.results[c]["xT"], np.float32
        ).T
    if _trace:
        return out, res
    return out
